# revision 54
# baseline (speedup 1.0000x reference)
"""Trainium2 Bass kernel for nn_Graphs (soft decision-graph probability propagation).

Reference math (G=4 graphs, B=128 batch, N=255 internal nodes, L=256 leaves,
F=512 features, J=8 jumps):
  b  = sigmoid(x @ W_g^T + bias_g)                  (per graph: B x N)
  M0 = softmax(M_left, axis=dest), M1 = softmax(M_right, axis=dest)
  q  = [b*(M1-M0)+M0 | leaf-identity]               (per (g,batch): 511x511)
  prob <- q @ prob, J times, starting from e0; return leaf probs.

Key restructure: q is never materialized. With u = prob[internal] and
v = b * u, one jump is
  prob_new = M0 @ u + (M1-M0) @ v   (+ leaf mass preserved)
Leaf rows only ever accumulate, so they live in a persistent PSUM
accumulator across all 8 jumps.  Jump 0's state is the one-hot e0, so it
collapses to K=1 outer-product matmuls against node-0 rows.

Softmax denominators are folded into the per-jump state scaling:
  M0n@u + (M1n-M0n)@(b*u) = E0@(r0*(1-b)*u) + E1@(r1*b*u)
with E = exp(M^T) raw and r = 1/rowsum.  The (1-b),b pair ("cb") comes
from the W-matmul track; rowsum reciprocals ("rpack") merge in with one
tensor_tensor per half.

Everything on the wire and in the matmul operands is fp16: f32r matmuls
run at 4 cycles/row when the moving dim is <256 (the N=64 internal jump
matmuls), while fp16 runs at 1 cycle/row and gets fast weight load; it
also halves the input DMA bytes.  Accumulation stays fp32 in PSUM.
Because the scaled state (r*c*u ~ 1e-6..1e-10) would sit in fp16's
subnormal range, a 2^16 scale is injected once at jump 0 (c256) -- the
recurrence preserves it (pq = 2^16*u for every jump) -- and the output
copy multiplies by 2^-16.  Expected outputs are ~3e-3..5e-3 so the
fp16 path keeps max relative error ~1e-3, well inside the 2e-2 gate.

Sharding: 8 cores = (graph g = core//2) x (batch half h = core%2, 64 rows).
No cross-core communication. Host pre-transposes/pads inputs so the device
does zero transposes:
  - m0/m1 (128,1024) fp16: M^T halves packed [mlt_h | mrt_h] with source
    node on partitions (pad row = 0), destination on free dim, leaf
    destinations at cols 256..511 of each 512-block (col 255 = -1e4 pad
    -> exp = 0). Softmax over dest = free-dim reduce.  m0 carries the
    t=0 halves that gate jump 0.
  - wx (128,1280) fp16: the four [W_g^T | x_half^T] K-tiles side by side.
  - biasp (1,320) fp16: [bias_h0 | bias_h1 | ones] row for K=1 matmuls.
Output per core: (64,256) fp16 batch-major leaf probs still carrying the
2^16 scale; host unscales, assembles to (B,L,G), applies the interval clamp.
"""

import numpy as np

G, B, N, L, F, J = 4, 128, 255, 256, 512, 8
BH = B // 2  # 64 batch rows per core
NCORES = 8
NEG = np.float32(-1e4)
SCALE = 65536.0

_CACHE = {}


def _build_program():
    import concourse.mybir as mybir
    from concourse import bacc
    from concourse.tile import TileContext

    f32 = mybir.dt.float32
    f16 = mybir.dt.float16
    bf16 = mybir.dt.bfloat16
    AF = mybir.ActivationFunctionType

    def rmm(out, lhsT, rhs, **kw):
        nc.tensor.matmul(out, lhsT, rhs, **kw)

    nc = bacc.Bacc(None)
    # m0 = [mlt_h0 | mrt_h0], m1 = [mlt_h1 | mrt_h1]: the t=0 halves gate
    # jump 0 (via rpack[0]/c256), so they ship in the first transfer.
    p_m0 = nc.declare_dram_parameter("m0", [128, 1024], f16, isOutput=False)
    p_m1 = nc.declare_dram_parameter("m1", [128, 1024], f16, isOutput=False)
    p_wx = nc.declare_dram_parameter("wx", [128, 4 * (256 + BH)], f16, isOutput=False)
    p_bias = nc.declare_dram_parameter("biasp", [1, 256 + BH], f16, isOutput=False)
    p_out = nc.declare_dram_parameter("out", [BH, 256], f16, isOutput=True)

    with TileContext(nc) as tc:
        with (
            tc.tile_pool(name="consts", bufs=1) as consts,
            tc.tile_pool(name="work", bufs=2) as work,
            tc.tile_pool(name="state", bufs=2) as state,
            tc.tile_pool(name="psum", bufs=2, space="PSUM") as psum,
            tc.tile_pool(name="psum_acc", bufs=1, space="PSUM") as psum_acc,
        ):
            # ---- PE warm-up stream ----
            # Dense dummy bf16 N=512 matmuls keep the PE HAM clock-gate at
            # 8/8 from kernel start until the jump loop takes over; a single
            # >~2us PE-idle gap re-throttles to 4/8 for the REST of the
            # kernel (observed on v1/v2 traces), so coverage must be gapless.
            wsc = consts.tile([128, 128], bf16, tag="wsc", name="wsc")
            rsc = consts.tile([128, 512], bf16, tag="rsc", name="rsc")
            nc.gpsimd.memset(wsc[:], 0.0)
            nc.gpsimd.memset(rsc[:], 0.0)
            pwarm = psum_acc.tile([128, 512], f32, tag="pwarm", name="pwarm")

            def warm(n):
                for _ in range(n):
                    nc.tensor.matmul(pwarm[:], wsc[:], rsc[:], start=True, stop=True)

            # ---- input DMAs ----
            # fp16 on the wire, consolidated into one large transfer per
            # queue (per-queue FIFO order is completion order; small DMAs
            # each pay a ~2us HBM-receipt latency and round-robin against
            # each other).  mlt alone on the sync queue lands first -> exp
            # chain starts earliest; wx leads the scalar queue so the
            # b-track's small eb exps can fill ACT gaps between big exps.
            m0all = consts.tile([128, 1024], f16, tag="m0all", name="m0all")
            m1all = consts.tile([128, 1024], f16, tag="m1all", name="m1all")
            wxall = consts.tile([128, 4 * (256 + BH)], f16, tag="wxall", name="wxall")
            nc.sync.dma_start(m0all[:], p_m0[:, :])
            nc.scalar.dma_start(wxall[:], p_wx[:, :])
            # bias row: [bias_h0(128) | bias_h1(128) | ones(64)] on one
            # partition -- contracted into pb via K=1 matmuls
            bias = consts.tile([1, 256 + BH], f16, tag="bias", name="bias")
            nc.gpsimd.dma_start(bias[:], p_bias[:, :])
            # m1 rides the otherwise-idle SWDGE queue: on sync it would
            # serialize behind m0's ~2.8us stream+receipt
            nc.gpsimd.dma_start(m1all[:], p_m1[:, :])

            warm(7)

            # ---- exp + row sums, lazily normalized ----
            # e0all = [exp(mlt_h0) | exp(mrt_h0)] comes from ONE mega ACT op
            # over m0all (saves an exp's ~350-cycle fixed cost plus two
            # accumulator reads on the serial ACT queue), with both row sums
            # via DVE reduces -- the Vector engine is idle this early.  The
            # t=1 tiles keep per-tile exps with ACT accumulator reads: at
            # the tail of the chain the 281ns read beats a 690ns DVE reduce.
            # rpack[t] cols = 1/rowsum for E0/E1.  Jump 0 only needs c256
            # (rpack[0] alone), so the jump loop launches before the t=1
            # exps/reciprocals finish.
            e0all = consts.tile([128, 1024], f16, tag="e0all", name="e0all")
            e1all = consts.tile([128, 1024], f16, tag="e1all", name="e1all")

            def EL(t, a, b, p=None):
                src = e0all if t == 0 else e1all
                return src[0:1, a:b] if p == 0 else src[:, a:b]

            def ER(t, a, b, p=None):
                src = e0all if t == 0 else e1all
                return src[0:1, 512 + a:512 + b] if p == 0 else src[:, 512 + a:512 + b]

            rpack = [consts.tile([128, 2], f32, tag=f"rpack{t}", name=f"rpack{t}") for t in range(2)]
            c01 = [consts.tile([128, 2, BH], f16, tag=f"c01{t}", name=f"c01{t}") for t in range(2)]
            c256 = consts.tile([1, 2, BH], f16, tag="c256", name="c256")
            cb = consts.tile([128, 2, 2, BH], f32, tag="cb", name="cb")  # [node, t, (1-b|b), batch]
            mult = mybir.AluOpType.mult
            AX = mybir.AxisListType
            add = mybir.AluOpType.add

            def c01_of(t):
                return nc.vector.tensor_tensor(
                    out=c01[t][:], in0=cb[:, t, :, :],
                    in1=rpack[t][:, :, None].broadcast_to([128, 2, BH]), op=mult)

            # the ACT accumulator sums the whole 1024-row (= el0sum + er0sum);
            # one DVE reduce recovers el0sum and a subtract gives er0sum --
            # one 690ns reduce instead of two on the c256 critical path
            ps_tot = work.tile([128, 1], f32, tag="ps", name="ps_tot")
            nc.scalar.activation(e0all[:], m0all[:], AF.Exp, accum_out=ps_tot[:])
            ps0 = work.tile([128, 1], f32, tag="ps", name="ps00")
            nc.vector.tensor_reduce(ps0[:], e0all[:, 0:512], axis=AX.X, op=add)
            ps2 = work.tile([128, 1], f32, tag="ps2", name="ps01")
            nc.vector.tensor_tensor(out=ps2[:], in0=ps_tot[:], in1=ps0[:],
                                    op=mybir.AluOpType.subtract)
            nc.vector.reciprocal(rpack[0][:, 0:1], ps0[:])
            nc.vector.reciprocal(rpack[0][:, 1:2], ps2[:])

            # ---- b-track: b = sigmoid(W @ x^T + bias) via exp ----
            # Both node-tile halves share one PSUM bank (one accumulation
            # group; the bank-wide has_written clear happens only on the
            # very first matmul).  The bias rides as two K=1 matmuls against
            # a ones-row, so a single bias-free eb exp covers both halves.
            WK = 256 + BH
            pb = psum.tile([128, 2, BH], f32, tag="pb", name="pb")
            for k in range(4):
                for mh in range(2):
                    rmm(
                        pb[:, mh, :], wxall[:, k * WK + mh * 128:k * WK + (mh + 1) * 128],
                        wxall[:, k * WK + 256:k * WK + 256 + BH],
                        start=(k == 0 and mh == 0), stop=False,
                    )
            for mh in range(2):
                rmm(pb[:, mh, :], bias[0:1, mh * 128:(mh + 1) * 128],
                    bias[0:1, 256:256 + BH], start=False, stop=(mh == 1))
            # b = sigmoid(z) = 0.5 + 0.5*tanh(z/2): tanh shares the exp ACT
            # table set (no extra table load) and the two tensor_scalar ops
            # replace an add + DVE reciprocal (~1us at FD=128: the iterative
            # divide runs 8 cycles/element) + mul on the c256 critical path.
            th = work.tile([128, 2, BH], f32, tag="th", name="th")
            nc.scalar.activation(th[:], pb[:], AF.Tanh, scale=0.5)
            nc.vector.tensor_scalar(
                out=cb[:, :, 1, :], in0=th[:], scalar1=0.5, scalar2=0.5,
                op0=mult, op1=mybir.AluOpType.add)
            nc.vector.tensor_scalar(
                out=cb[:, :, 0, :], in0=th[:], scalar1=-0.5, scalar2=0.5,
                op0=mult, op1=mybir.AluOpType.add)
            # 2^16-scaled node-0 rows for jump 0 (seeds the scale into the
            # whole chain; see module docstring).
            c01_of(0)
            nc.vector.tensor_scalar_mul(c256[:], c01[0][0:1, :, :], SCALE)

            # same mega-exp + subtract as the t=0 tiles: one ACT op instead
            # of two exps + two accumulator reads on the serial ACT queue,
            # and the el1/er1 data is ready ~0.6us earlier for jump 1
            ps1_tot = work.tile([128, 1], f32, tag="ps", name="ps1_tot")
            nc.scalar.activation(e1all[:], m1all[:], AF.Exp, accum_out=ps1_tot[:])
            ps10 = work.tile([128, 1], f32, tag="ps", name="ps10")
            nc.vector.tensor_reduce(ps10[:], e1all[:, 0:512], axis=AX.X, op=add)
            ps12 = work.tile([128, 1], f32, tag="ps2", name="ps11")
            nc.vector.tensor_tensor(out=ps12[:], in0=ps1_tot[:], in1=ps10[:],
                                    op=mybir.AluOpType.subtract)
            nc.vector.reciprocal(rpack[1][:, 0:1], ps10[:])
            nc.vector.reciprocal(rpack[1][:, 1:2], ps12[:])
            c01_1_inst = c01_of(1)

            # ---- jump loop ----
            # State is the scaled pair (up = c0*u, v = c1*u) in fp16,
            # recomputed each jump from the previous jump's PSUM.
            pleaf = psum_acc.tile([BH, 256], f32, tag="pl", name="pl")

            # Jump 0: state is one-hot e0 -> K=1 outer products of the
            # node-0 rows of E0/E1 with the scaled node-0 c01 rows.
            pq = [psum.tile([128, BH], f32, tag=f"pq{mt}", name=f"pq{mt}") for mt in range(2)]
            for mt in range(2):
                a = mt * 128
                rmm(pq[mt][:], EL(0, a, a + 128, p=0), c256[0:1, 0, :], start=True, stop=False)
                rmm(pq[mt][:], ER(0, a, a + 128, p=0), c256[0:1, 1, :], start=False, stop=True)
            rmm(pleaf[:], c256[0:1, 0, :], EL(0, 256, 512, p=0), start=True, stop=False)
            rmm(pleaf[:], c256[0:1, 1, :], ER(0, 256, 512, p=0), start=False, stop=False)
            # fills the PE while jump 1 waits for upv (c01[1] arrives with
            # the last exp's row sums)
            warm(3)

            # Jumps 1..7
            from concourse.tile_rust import add_dep_helper
            for j in range(1, J):
                upv = [state.tile([128, 2, BH], f16, tag=f"upv{t}", name=f"upv{t}") for t in range(2)]
                for t in range(2):
                    tt = nc.vector.tensor_tensor(
                        out=upv[t][:], in0=c01[t][:],
                        in1=pq[t][:, None, :].broadcast_to([128, 2, BH]), op=mult)
                    if j == 1:
                        # order jump 1's upv TTs after c01(1) on the DVE
                        # queue: the scheduler otherwise hoists them ahead,
                        # and (stalled on the PE's pq semaphore) they
                        # head-of-line-block the rpack[1] reciprocals that
                        # c01(1) needs.  Cannot delay jump 1's completion --
                        # that is gated by upv[1] <- c01(1) regardless.
                        add_dep_helper(tt.ins, c01_1_inst.ins, sync=False,
                                       reason="keep c01(1) ahead of jump-1 upv on DVE")
                up = [upv[t][:, 0] for t in range(2)]
                v = [upv[t][:, 1] for t in range(2)]
                if j < J - 1:
                    pq = [psum.tile([128, BH], f32, tag=f"pq{mt}", name=f"pq{mt}") for mt in range(2)]
                    for mt in range(2):
                        a = mt * 128
                        # t=0 operands first: these only need upv[0], so the
                        # group can start while upv[1] is still computing
                        rmm(pq[mt][:], EL(0, a, a + 128), up[0], start=True, stop=False)
                        rmm(pq[mt][:], ER(0, a, a + 128), v[0], start=False, stop=False)
                        rmm(pq[mt][:], EL(1, a, a + 128), up[1], start=False, stop=False)
                        rmm(pq[mt][:], ER(1, a, a + 128), v[1], start=False, stop=True)
                # high-duty dummies keep the HAM activity monitor fed (the
                # fp16 N=64 stream alone reads as near-idle to it; removing
                # ALL of them measured +3us from the mid-loop re-throttle).
                # Every other jump suffices: the low-duty stretch between
                # dummies (~2.1us) stays under the ~3.4us MID idle window.
                if j % 2 == 1:
                    warm(1)
                last = j == J - 1
                rmm(pleaf[:], up[0], EL(0, 256, 512), start=False, stop=False)
                rmm(pleaf[:], v[0], ER(0, 256, 512), start=False, stop=False)
                rmm(pleaf[:], up[1], EL(1, 256, 512), start=False, stop=False)
                rmm(pleaf[:], v[1], ER(1, 256, 512), start=False, stop=last)

            # ---- output ----
            # fp16, still carrying the 2^16 scale -- the host unscales while
            # assembling (free), and outputs ~3e-3 * 2^16 ~ 200 sit in
            # fp16's sweet spot.  Two halves on the two HWDGE queues so the
            # PSUM->SBUF copies and the DMA receipt latencies overlap.
            o = work.tile([BH, 256], f16, tag="o", name="o")
            nc.vector.tensor_copy(o[:, 0:128], pleaf[:, 0:128])
            nc.sync.dma_start(p_out[:, 0:128], o[:, 0:128])
            nc.vector.tensor_copy(o[:, 128:256], pleaf[:, 128:256])
            nc.scalar.dma_start(p_out[:, 128:256], o[:, 128:256])

    nc.finalize()
    return nc


def _get_program():
    if "nc" not in _CACHE:
        _CACHE["nc"] = _build_program()
    return _CACHE["nc"]


def _prep_inputs(x, W, bias, M_left, M_right):
    """Host-side shard + layout prep. Core c -> graph c//2, batch half c%2."""
    in_maps = []
    mlt_g, mrt_g, wt_g, bias_g = [], [], [], []
    for g in range(G):
        mlt = np.zeros((256, 512), np.float32)
        mrt = np.zeros((256, 512), np.float32)
        tl = np.ascontiguousarray(M_left[g].T)   # (255, 511)
        tr = np.ascontiguousarray(M_right[g].T)
        for dst, src in ((mlt, tl), (mrt, tr)):
            dst[0:255, 0:255] = src[:, 0:255]
            dst[0:255, 256:512] = src[:, 255:511]
            dst[0:255, 255] = NEG
        # m0 = [mlt_h0 | mrt_h0], m1 = [mlt_h1 | mrt_h1]: one DMA each,
        # t=0 halves (which gate jump 0) in the first transfer
        mlt_g.append(np.concatenate([mlt[0:128], mrt[0:128]], axis=1).astype(np.float16))
        mrt_g.append(np.concatenate([mlt[128:256], mrt[128:256]], axis=1).astype(np.float16))
        wt = np.zeros((512, 256), np.float32)
        wt[:, 0:255] = W[g].T
        wt_g.append(wt)
        # bias row for the K=1 matmuls: [bias_h0 | bias_h1 | ones]
        bp = np.zeros((1, 256 + BH), np.float32)
        bp[0, 0:128] = bias[g][0:128]
        bp[0, 128:255] = bias[g][128:255]
        bp[0, 256:256 + BH] = 1.0
        bias_g.append(bp.astype(np.float16))
    xt_h = [np.ascontiguousarray(x[h * BH:(h + 1) * BH].T) for h in range(2)]
    for c in range(NCORES):
        g, h = c // 2, c % 2
        wx = np.concatenate([wt_g[g], xt_h[h]], axis=1)  # (512, 320)
        # pack the 4 K-tiles side by side: (128, 1280), one DMA
        wx = np.concatenate([wx[k * 128:(k + 1) * 128] for k in range(4)], axis=1)
        in_maps.append({
            "m0": mlt_g[g], "m1": mrt_g[g],
            "wx": np.ascontiguousarray(wx.astype(np.float16)),
            "biasp": bias_g[g],
        })
    return in_maps


def _assemble(results):
    eps = np.float32(1e-5)
    ret = np.empty((B, L, G), np.float32)
    inv = np.float32(1.0 / SCALE)
    for c in range(NCORES):
        g, h = c // 2, c % 2
        ret[h * BH:(h + 1) * BH, :, g] = results[c]["out"].astype(np.float32) * inv
    ret = np.where(ret > 0.0, ret, eps)
    ret = np.where(ret < 1.0, ret, np.float32(1.0) - eps)
    return ret.astype(np.float32)


def run_on_device(in_maps, trace=False, **kw):
    from concourse.bass_utils import run_bass_kernel_spmd
    nc = _get_program()
    return run_bass_kernel_spmd(nc, in_maps, list(range(NCORES)), trace=trace, **kw)


def kernel(x, W, bias, M_left, M_right):
    in_maps = _prep_inputs(
        np.asarray(x, np.float32), np.asarray(W, np.float32),
        np.asarray(bias, np.float32), np.asarray(M_left, np.float32),
        np.asarray(M_right, np.float32),
    )
    res = run_on_device(in_maps)
    return _assemble(res.results)


# revision 55
# speedup vs baseline: 1.1308x; 1.1308x over previous
"""Trainium2 Bass kernel for nn_Graphs (soft decision-graph probability propagation).

Reference math (G=4 graphs, B=128 batch, N=255 internal nodes, L=256 leaves,
F=512 features, J=8 jumps):
  b  = sigmoid(x @ W_g^T + bias_g)                  (per graph: B x N)
  M0 = softmax(M_left, axis=dest), M1 = softmax(M_right, axis=dest)
  q  = [b*(M1-M0)+M0 | leaf-identity]               (per (g,batch): 511x511)
  prob <- q @ prob, J times, starting from e0; return leaf probs.

Key restructure: q is never materialized. With u = prob[internal] and
v = b * u, one jump is
  prob_new = M0 @ u + (M1-M0) @ v   (+ leaf mass preserved)
Leaf rows only ever accumulate, so they live in a persistent PSUM
accumulator across all 8 jumps.  Jump 0's state is the one-hot e0, so it
collapses to K=1 outer-product matmuls against node-0 rows.

Softmax denominators are folded into the per-jump state scaling:
  M0n@u + (M1n-M0n)@(b*u) = E0@(r0*(1-b)*u) + E1@(r1*b*u)
with E = exp(M^T) raw and r = 1/rowsum.  The (1-b),b pair ("cb") comes
from the W-matmul track; rowsum reciprocals ("rpack") merge in with one
tensor_tensor per half.

Everything on the wire and in the matmul operands is fp16: f32r matmuls
run at 4 cycles/row when the moving dim is <256 (the N=64 internal jump
matmuls), while fp16 runs at 1 cycle/row and gets fast weight load; it
also halves the input DMA bytes.  Accumulation stays fp32 in PSUM.
Because the scaled state (r*c*u ~ 1e-6..1e-10) would sit in fp16's
subnormal range, a 2^16 scale is injected once at jump 0 (c256) -- the
recurrence preserves it (pq = 2^16*u for every jump) -- and the output
copy multiplies by 2^-16.  Expected outputs are ~3e-3..5e-3 so the
fp16 path keeps max relative error ~1e-3, well inside the 2e-2 gate.

Sharding: 8 cores = (graph g = core//2) x (batch half h = core%2, 64 rows).
No cross-core communication. Host pre-transposes/pads inputs so the device
does zero transposes:
  - m0/m1 (128,1024) fp16: M^T halves packed [mlt_h | mrt_h] with source
    node on partitions (pad row = 0), destination on free dim, leaf
    destinations at cols 256..511 of each 512-block (col 255 = -1e4 pad
    -> exp = 0). Softmax over dest = free-dim reduce.  m0 carries the
    t=0 halves that gate jump 0.
  - wx (128,1280) fp16: the four [W_g^T | x_half^T] K-tiles side by side.
  - biasp (1,320) fp16: [bias_h0 | bias_h1 | ones] row for K=1 matmuls.
Output per core: (64,256) fp16 batch-major leaf probs still carrying the
2^16 scale; host unscales, assembles to (B,L,G), applies the interval clamp.
"""

import numpy as np

G, B, N, L, F, J = 4, 128, 255, 256, 512, 8
BH = B // 2  # 64 batch rows per core
NCORES = 8
NEG = np.float32(-1e4)
SCALE = 65536.0

_CACHE = {}


def _build_program():
    import concourse.mybir as mybir
    from concourse import bacc
    from concourse.tile import TileContext

    f32 = mybir.dt.float32
    f16 = mybir.dt.float16
    bf16 = mybir.dt.bfloat16
    AF = mybir.ActivationFunctionType

    def rmm(out, lhsT, rhs, **kw):
        nc.tensor.matmul(out, lhsT, rhs, **kw)

    nc = bacc.Bacc(None)
    # m0 = [mlt_h0 | mrt_h0], m1 = [mlt_h1 | mrt_h1]: the t=0 halves gate
    # jump 0 (via rpack[0]/c256), so they ship in the first transfer.
    p_m0 = nc.declare_dram_parameter("m0", [128, 1024], f16, isOutput=False)
    p_m1 = nc.declare_dram_parameter("m1", [128, 1024], f16, isOutput=False)
    p_wx = nc.declare_dram_parameter("wx", [128, 4 * (256 + BH)], f16, isOutput=False)
    p_bias = nc.declare_dram_parameter("biasp", [1, 256 + BH], f16, isOutput=False)
    p_out = nc.declare_dram_parameter("out", [BH, 256], f16, isOutput=True)

    with TileContext(nc) as tc:
        with (
            tc.tile_pool(name="consts", bufs=1) as consts,
            tc.tile_pool(name="work", bufs=2) as work,
            tc.tile_pool(name="state", bufs=2) as state,
            tc.tile_pool(name="psum", bufs=2, space="PSUM") as psum,
            tc.tile_pool(name="psum_acc", bufs=1, space="PSUM") as psum_acc,
        ):
            # ---- PE warm-up stream ----
            # Dense dummy bf16 N=512 matmuls keep the PE HAM clock-gate at
            # 8/8 from kernel start until the jump loop takes over; a single
            # >~2us PE-idle gap re-throttles to 4/8 for the REST of the
            # kernel (observed on v1/v2 traces), so coverage must be gapless.
            wsc = consts.tile([128, 128], bf16, tag="wsc", name="wsc")
            rsc = consts.tile([128, 512], bf16, tag="rsc", name="rsc")
            nc.gpsimd.memset(wsc[:], 0.0)
            nc.gpsimd.memset(rsc[:], 0.0)
            pwarm = psum_acc.tile([128, 512], f32, tag="pwarm", name="pwarm")

            def warm(n):
                for _ in range(n):
                    nc.tensor.matmul(pwarm[:], wsc[:], rsc[:], start=True, stop=True)

            # ---- input DMAs ----
            # fp16 on the wire, consolidated into one large transfer per
            # queue (per-queue FIFO order is completion order; small DMAs
            # each pay a ~2us HBM-receipt latency and round-robin against
            # each other).  mlt alone on the sync queue lands first -> exp
            # chain starts earliest; wx leads the scalar queue so the
            # b-track's small eb exps can fill ACT gaps between big exps.
            m0all = consts.tile([128, 1024], f16, tag="m0all", name="m0all")
            m1all = consts.tile([128, 1024], f16, tag="m1all", name="m1all")
            wxall = consts.tile([128, 4 * (256 + BH)], f16, tag="wxall", name="wxall")
            nc.sync.dma_start(m0all[:], p_m0[:, :])
            nc.scalar.dma_start(wxall[:], p_wx[:, :])
            # bias row: [bias_h0(128) | bias_h1(128) | ones(64)] on one
            # partition -- contracted into pb via K=1 matmuls
            bias = consts.tile([1, 256 + BH], f16, tag="bias", name="bias")
            nc.gpsimd.dma_start(bias[:], p_bias[:, :])
            # m1 rides the otherwise-idle SWDGE queue: on sync it would
            # serialize behind m0's ~2.8us stream+receipt
            nc.gpsimd.dma_start(m1all[:], p_m1[:, :])

            warm(7)

            # ---- exp + row sums, lazily normalized ----
            # e0all = [exp(mlt_h0) | exp(mrt_h0)] comes from ONE mega ACT op
            # over m0all (saves an exp's ~350-cycle fixed cost plus two
            # accumulator reads on the serial ACT queue), with both row sums
            # via DVE reduces -- the Vector engine is idle this early.  The
            # t=1 tiles keep per-tile exps with ACT accumulator reads: at
            # the tail of the chain the 281ns read beats a 690ns DVE reduce.
            # rpack[t] cols = 1/rowsum for E0/E1.  Jump 0 only needs c256
            # (rpack[0] alone), so the jump loop launches before the t=1
            # exps/reciprocals finish.
            e0all = consts.tile([128, 1024], f16, tag="e0all", name="e0all")
            e1all = consts.tile([128, 1024], f16, tag="e1all", name="e1all")

            def EL(t, a, b, p=None):
                src = e0all if t == 0 else e1all
                return src[0:1, a:b] if p == 0 else src[:, a:b]

            def ER(t, a, b, p=None):
                src = e0all if t == 0 else e1all
                return src[0:1, 512 + a:512 + b] if p == 0 else src[:, 512 + a:512 + b]

            rpack = [consts.tile([128, 2], f32, tag=f"rpack{t}", name=f"rpack{t}") for t in range(2)]
            c01 = [consts.tile([128, 2, BH], f16, tag=f"c01{t}", name=f"c01{t}") for t in range(2)]
            c256 = consts.tile([1, 2, BH], f16, tag="c256", name="c256")
            cb = consts.tile([128, 2, 2, BH], f32, tag="cb", name="cb")  # [node, t, (1-b|b), batch]
            mult = mybir.AluOpType.mult
            AX = mybir.AxisListType
            add = mybir.AluOpType.add

            def c01_of(t):
                return nc.vector.tensor_tensor(
                    out=c01[t][:], in0=cb[:, t, :, :],
                    in1=rpack[t][:, :, None].broadcast_to([128, 2, BH]), op=mult)

            # the ACT accumulator sums the whole 1024-row (= el0sum + er0sum);
            # one DVE reduce recovers el0sum and a subtract gives er0sum --
            # one 690ns reduce instead of two on the c256 critical path
            ps_tot = work.tile([128, 1], f32, tag="ps", name="ps_tot")
            nc.scalar.activation(e0all[:], m0all[:], AF.Exp, accum_out=ps_tot[:])
            ps0 = work.tile([128, 1], f32, tag="ps", name="ps00")
            nc.vector.tensor_reduce(ps0[:], e0all[:, 0:512], axis=AX.X, op=add)
            ps2 = work.tile([128, 1], f32, tag="ps2", name="ps01")
            nc.vector.tensor_tensor(out=ps2[:], in0=ps_tot[:], in1=ps0[:],
                                    op=mybir.AluOpType.subtract)
            nc.vector.reciprocal(rpack[0][:, 0:1], ps0[:])
            nc.vector.reciprocal(rpack[0][:, 1:2], ps2[:])

            # ---- b-track: b = sigmoid(W @ x^T + bias) via exp ----
            # Both node-tile halves share one PSUM bank (one accumulation
            # group; the bank-wide has_written clear happens only on the
            # very first matmul).  The bias rides as two K=1 matmuls against
            # a ones-row, so a single bias-free eb exp covers both halves.
            WK = 256 + BH
            pb = psum.tile([128, 2, BH], f32, tag="pb", name="pb")
            for k in range(4):
                for mh in range(2):
                    rmm(
                        pb[:, mh, :], wxall[:, k * WK + mh * 128:k * WK + (mh + 1) * 128],
                        wxall[:, k * WK + 256:k * WK + 256 + BH],
                        start=(k == 0 and mh == 0), stop=False,
                    )
            for mh in range(2):
                rmm(pb[:, mh, :], bias[0:1, mh * 128:(mh + 1) * 128],
                    bias[0:1, 256:256 + BH], start=False, stop=(mh == 1))
            # b = sigmoid(z) = 0.5 + 0.5*tanh(z/2): tanh shares the exp ACT
            # table set (no extra table load) and the two tensor_scalar ops
            # replace an add + DVE reciprocal (~1us at FD=128: the iterative
            # divide runs 8 cycles/element) + mul on the c256 critical path.
            th = work.tile([128, 2, BH], f32, tag="th", name="th")
            nc.scalar.activation(th[:], pb[:], AF.Tanh, scale=0.5)
            nc.vector.tensor_scalar(
                out=cb[:, :, 1, :], in0=th[:], scalar1=0.5, scalar2=0.5,
                op0=mult, op1=mybir.AluOpType.add)
            nc.vector.tensor_scalar(
                out=cb[:, :, 0, :], in0=th[:], scalar1=-0.5, scalar2=0.5,
                op0=mult, op1=mybir.AluOpType.add)
            # 2^16-scaled node-0 rows for jump 0 (seeds the scale into the
            # whole chain; see module docstring).
            c01_of(0)
            nc.vector.tensor_scalar_mul(c256[:], c01[0][0:1, :, :], SCALE)

            # same mega-exp + subtract as the t=0 tiles: one ACT op instead
            # of two exps + two accumulator reads on the serial ACT queue,
            # and the el1/er1 data is ready ~0.6us earlier for jump 1
            ps1_tot = work.tile([128, 1], f32, tag="ps", name="ps1_tot")
            nc.scalar.activation(e1all[:], m1all[:], AF.Exp, accum_out=ps1_tot[:])
            ps10 = work.tile([128, 1], f32, tag="ps", name="ps10")
            nc.vector.tensor_reduce(ps10[:], e1all[:, 0:512], axis=AX.X, op=add)
            ps12 = work.tile([128, 1], f32, tag="ps2", name="ps11")
            nc.vector.tensor_tensor(out=ps12[:], in0=ps1_tot[:], in1=ps10[:],
                                    op=mybir.AluOpType.subtract)
            nc.vector.reciprocal(rpack[1][:, 0:1], ps10[:])
            nc.vector.reciprocal(rpack[1][:, 1:2], ps12[:])
            c01_1_inst = c01_of(1)

            # ---- jump loop ----
            # State is the scaled pair (up = c0*u, v = c1*u) in fp16,
            # recomputed each jump from the previous jump's PSUM.
            pleaf = psum_acc.tile([BH, 256], f32, tag="pl", name="pl")

            # Jump 0: state is one-hot e0 -> K=1 outer products of the
            # node-0 rows of E0/E1 with the scaled node-0 c01 rows.
            pq = [psum.tile([128, BH], f32, tag=f"pq{mt}", name=f"pq{mt}") for mt in range(2)]
            for mt in range(2):
                a = mt * 128
                rmm(pq[mt][:], EL(0, a, a + 128, p=0), c256[0:1, 0, :], start=True, stop=False)
                rmm(pq[mt][:], ER(0, a, a + 128, p=0), c256[0:1, 1, :], start=False, stop=True)
            rmm(pleaf[:], c256[0:1, 0, :], EL(0, 256, 512, p=0), start=True, stop=False)
            rmm(pleaf[:], c256[0:1, 1, :], ER(0, 256, 512, p=0), start=False, stop=False)
            # fills the PE while jump 1 waits for upv (c01[1] arrives with
            # the last exp's row sums)
            warm(3)

            # Jumps 1..7
            from concourse.tile_rust import add_dep_helper
            for j in range(1, J):
                upv = [state.tile([128, 2, BH], f16, tag=f"upv{t}", name=f"upv{t}") for t in range(2)]
                for t in range(2):
                    tt = nc.vector.tensor_tensor(
                        out=upv[t][:], in0=c01[t][:],
                        in1=pq[t][:, None, :].broadcast_to([128, 2, BH]), op=mult)
                    if j == 1:
                        # order jump 1's upv TTs after c01(1) on the DVE
                        # queue: the scheduler otherwise hoists them ahead,
                        # and (stalled on the PE's pq semaphore) they
                        # head-of-line-block the rpack[1] reciprocals that
                        # c01(1) needs.  Cannot delay jump 1's completion --
                        # that is gated by upv[1] <- c01(1) regardless.
                        add_dep_helper(tt.ins, c01_1_inst.ins, sync=False,
                                       reason="keep c01(1) ahead of jump-1 upv on DVE")
                up = [upv[t][:, 0] for t in range(2)]
                v = [upv[t][:, 1] for t in range(2)]
                if j < J - 1:
                    pq = [psum.tile([128, BH], f32, tag=f"pq{mt}", name=f"pq{mt}") for mt in range(2)]
                    for mt in range(2):
                        a = mt * 128
                        # t=0 operands first: these only need upv[0], so the
                        # group can start while upv[1] is still computing
                        rmm(pq[mt][:], EL(0, a, a + 128), up[0], start=True, stop=False)
                        rmm(pq[mt][:], ER(0, a, a + 128), v[0], start=False, stop=False)
                        rmm(pq[mt][:], EL(1, a, a + 128), up[1], start=False, stop=False)
                        rmm(pq[mt][:], ER(1, a, a + 128), v[1], start=False, stop=True)
                # one high-duty dummy per jump keeps the HAM activity monitor
                # fed (the fp16 N=64 stream alone reads as near-idle to it).
                # Trace-proven necessary at this density: removing all of
                # them measured +3us, and even every-other-jump re-throttled
                # to K=4/8 mid-loop (1.58us/jump cold vs 1.05 warm).
                warm(1)
                last = j == J - 1
                rmm(pleaf[:], up[0], EL(0, 256, 512), start=False, stop=False)
                rmm(pleaf[:], v[0], ER(0, 256, 512), start=False, stop=False)
                rmm(pleaf[:], up[1], EL(1, 256, 512), start=False, stop=False)
                rmm(pleaf[:], v[1], ER(1, 256, 512), start=False, stop=last)

            # ---- output ----
            # fp16, still carrying the 2^16 scale -- the host unscales while
            # assembling (free), and outputs ~3e-3 * 2^16 ~ 200 sit in
            # fp16's sweet spot.  Two halves on the two HWDGE queues so the
            # PSUM->SBUF copies and the DMA receipt latencies overlap.
            o = work.tile([BH, 256], f16, tag="o", name="o")
            nc.vector.tensor_copy(o[:, 0:128], pleaf[:, 0:128])
            nc.sync.dma_start(p_out[:, 0:128], o[:, 0:128])
            nc.vector.tensor_copy(o[:, 128:256], pleaf[:, 128:256])
            nc.scalar.dma_start(p_out[:, 128:256], o[:, 128:256])

    nc.finalize()
    return nc


def _get_program():
    if "nc" not in _CACHE:
        _CACHE["nc"] = _build_program()
    return _CACHE["nc"]


def _prep_inputs(x, W, bias, M_left, M_right):
    """Host-side shard + layout prep. Core c -> graph c//2, batch half c%2."""
    in_maps = []
    mlt_g, mrt_g, wt_g, bias_g = [], [], [], []
    for g in range(G):
        mlt = np.zeros((256, 512), np.float32)
        mrt = np.zeros((256, 512), np.float32)
        tl = np.ascontiguousarray(M_left[g].T)   # (255, 511)
        tr = np.ascontiguousarray(M_right[g].T)
        for dst, src in ((mlt, tl), (mrt, tr)):
            dst[0:255, 0:255] = src[:, 0:255]
            dst[0:255, 256:512] = src[:, 255:511]
            dst[0:255, 255] = NEG
        # m0 = [mlt_h0 | mrt_h0], m1 = [mlt_h1 | mrt_h1]: one DMA each,
        # t=0 halves (which gate jump 0) in the first transfer
        mlt_g.append(np.concatenate([mlt[0:128], mrt[0:128]], axis=1).astype(np.float16))
        mrt_g.append(np.concatenate([mlt[128:256], mrt[128:256]], axis=1).astype(np.float16))
        wt = np.zeros((512, 256), np.float32)
        wt[:, 0:255] = W[g].T
        wt_g.append(wt)
        # bias row for the K=1 matmuls: [bias_h0 | bias_h1 | ones]
        bp = np.zeros((1, 256 + BH), np.float32)
        bp[0, 0:128] = bias[g][0:128]
        bp[0, 128:255] = bias[g][128:255]
        bp[0, 256:256 + BH] = 1.0
        bias_g.append(bp.astype(np.float16))
    xt_h = [np.ascontiguousarray(x[h * BH:(h + 1) * BH].T) for h in range(2)]
    for c in range(NCORES):
        g, h = c // 2, c % 2
        wx = np.concatenate([wt_g[g], xt_h[h]], axis=1)  # (512, 320)
        # pack the 4 K-tiles side by side: (128, 1280), one DMA
        wx = np.concatenate([wx[k * 128:(k + 1) * 128] for k in range(4)], axis=1)
        in_maps.append({
            "m0": mlt_g[g], "m1": mrt_g[g],
            "wx": np.ascontiguousarray(wx.astype(np.float16)),
            "biasp": bias_g[g],
        })
    return in_maps


def _assemble(results):
    eps = np.float32(1e-5)
    ret = np.empty((B, L, G), np.float32)
    inv = np.float32(1.0 / SCALE)
    for c in range(NCORES):
        g, h = c // 2, c % 2
        ret[h * BH:(h + 1) * BH, :, g] = results[c]["out"].astype(np.float32) * inv
    ret = np.where(ret > 0.0, ret, eps)
    ret = np.where(ret < 1.0, ret, np.float32(1.0) - eps)
    return ret.astype(np.float32)


def run_on_device(in_maps, trace=False, **kw):
    from concourse.bass_utils import run_bass_kernel_spmd
    nc = _get_program()
    return run_bass_kernel_spmd(nc, in_maps, list(range(NCORES)), trace=trace, **kw)


def kernel(x, W, bias, M_left, M_right):
    in_maps = _prep_inputs(
        np.asarray(x, np.float32), np.asarray(W, np.float32),
        np.asarray(bias, np.float32), np.asarray(M_left, np.float32),
        np.asarray(M_right, np.float32),
    )
    res = run_on_device(in_maps)
    return _assemble(res.results)
